# revision 1
# baseline (speedup 1.0000x reference)
"""AngleLoss (HANDS17 bone-angle loss) on 8 TRN2 NeuronCores.

Math (per batch element b, bone pair (i0, i1)):
    v1 = pred[b, i0, :2] - pred[b, i1, :2]
    v2 = gt[b, i0, :2]   - gt[b, i1, :2]
    t  = |v1 . v2| / (|v1| |v2|)
    loss = mean over (b, pair) of (1 - t)

Strategy: pure data parallel over the batch. Each core streams its
65536-element batch shard (33 MB of f32) through SBUF in batch-major
layout [128 partitions, C*63 floats], computes bone vectors with four
strided tensor_sub gathers (bf16 out, [xy, pair, batch] layout so the
innermost AP run is the C-long batch dim), the quadratic forms and the
rsqrt via exp(-0.5*ln(den)) split across DVE and ACT, and reduces
t over the batch with a ones-vector matmul accumulated in PSUM.
Each core emits one partial sum; the host combines 8 scalars.

The HANDS17 pair list (i0 = [0 x5, chains], i1 = p+1) decomposes into
four affine gathers, so no indirect addressing is needed:
    slots  0- 4: u[0]        - u[1..5]           (broadcast, step 3)
    slots  5- 9: u[1..5]     - u[6,9,12,15,18]   (step 3, step 9)
    slots 10-14: u[6,9,..18] - u[7,10,..19]      (step 9, step 9)
    slots 15-19: u[7,10,.19] - u[8,11,..20]      (step 9, step 9)
Slot order is a permutation of the pair list, which is fine because the
result is a sum over pairs.
"""
import sys

sys.path.insert(0, "/opt/trn_rl_repo")

from contextlib import ExitStack

import numpy as np

import concourse.bass as bass
import concourse.tile as tile
from concourse import mybir
from concourse.bass_utils import run_bass_kernel_spmd

B, J, DCOORD = 524288, 21, 3
NCORES = 8
P = 128                      # SBUF partitions
F = J * DCOORD               # 63 floats per batch element
NPAIR = 20

f32 = mybir.dt.float32
bf16 = mybir.dt.bfloat16
AF = mybir.ActivationFunctionType


def _split_excess_waits(nc, max_waits: int = 1) -> int:
    """The staged neuronxcc rejects instructions with more than one
    semaphore wait. Same-engine instructions run in order, so excess
    waits move onto preceding NoOps on the same engine."""
    n_split = 0
    for b in nc.m.functions[0].blocks:
        insts = b.instructions
        out = []
        changed = False
        for inst in insts:
            si = getattr(inst, "sync_info", None)
            waits = list(si.on_wait) if si is not None and si.on_wait else []
            if len(waits) > max_waits:
                extra, keep = waits[:-max_waits], waits[-max_waits:]
                while extra:
                    grp, extra = extra[:max_waits], extra[max_waits:]
                    nop = mybir.InstNoOp(
                        name=f"I-waitsplit-{n_split}", engine=inst.engine
                    )
                    nop.sync_info = mybir.SyncInfo(on_wait=grp, on_update=[])
                    out.append(nop)
                    n_split += 1
                inst.sync_info = mybir.SyncInfo(
                    on_wait=keep, on_update=list(si.on_update)
                )
                changed = True
            out.append(inst)
        if changed:
            insts[:] = out
    return n_split


def _compact_uv(nc, src, dst, C: int, c0: int, c1: int):
    """dst[p, c0:c1, joint, xy] (bf16, [C, 21, 2], batch-major
    interleaved) = uv coords gathered from src [P, C*63] f32 batch range
    [c0, c1). Iteration order [c][joint][xy] gives a fully contiguous
    output and uv-pair-innermost reads, which keeps the DVE cast-copy at
    its 2x dual-port rate (measured 0.66 cyc/elem, same as a fully
    contiguous copy)."""
    src_ap = src[:].rearrange("p (c j k) -> p c j k", j=21, k=3)[:, 0 : c1 - c0, :, 0:2]
    nc.vector.tensor_copy(out=dst[:, c0:c1, :, :], in_=src_ap)


def _emit_bone_subs(nc, u, dst, C: int):
    """dst[p, c, slot, xy] = bone vectors, bf16, from the compacted
    u [P, C, 21, 2] bf16 tile. Every operand's innermost run is a
    4B-aligned unit-stride uv pair in bf16, so tensor_sub runs 2x_1P."""
    root = u[:, :, 0:1, :].broadcast_to([P, C, 5, 2])
    subs = [
        (0, root, u[:, :, 1:6, :]),
        (5, u[:, :, 1:6, :], u[:, :, 6:19:3, :]),
        (10, u[:, :, 6:19:3, :], u[:, :, 7:20:3, :]),
        (15, u[:, :, 7:20:3, :], u[:, :, 8:21:3, :]),
    ]
    for s0, in0, in1 in subs:
        nc.vector.tensor_sub(out=dst[:, :, s0 : s0 + 5, :], in0=in0, in1=in1)


def build_nc(tiles) -> bass.Bass:
    """One core's kernel. `tiles` is the list of per-tile batch counts C
    (batch elements per partition); total batch = P * sum(tiles).
    Small leading tiles shorten the initial DMA wait (ramp-up)."""
    BL = P * sum(tiles)
    nc = bass.Bass()
    x_ext = nc.declare_dram_parameter("jt_uvd_pred", [BL, F], f32, isOutput=False)
    g_ext = nc.declare_dram_parameter("jt_uvd_gt", [BL, F], f32, isOutput=False)
    out_ext = nc.declare_dram_parameter("out", [1, 1], f32, isOutput=True)
    NFMAX = NPAIR * max(tiles)

    with tile.TileContext(nc) as tc, ExitStack() as ctx:
        ins_pool = ctx.enter_context(tc.tile_pool(name="ins", bufs=2))
        mid_pool = ctx.enter_context(tc.tile_pool(name="mid", bufs=2))
        small_pool = ctx.enter_context(tc.tile_pool(name="small", bufs=2))
        const_pool = ctx.enter_context(tc.tile_pool(name="const", bufs=1))
        psum_pool = ctx.enter_context(tc.tile_pool(name="psum", bufs=1, space="PSUM"))

        ones = const_pool.tile([P, 1], bf16)
        nc.vector.memset(ones[:], 1.0)
        # bf16-rounded inputs can collide -> exact-zero bones -> den=0;
        # ln(den+eps) keeps those pairs at t = |0|*huge = 0 instead of NaN
        eps = const_pool.tile([P, 1], f32)
        nc.vector.memset(eps[:], 1e-30)

        # PSUM accumulators for the batch reduction, <=512 f32 per bank.
        # Zeroed up front so variable-size tiles can all accumulate with
        # start=False (a start=True reset would only cover the columns of
        # whichever tile happened to write first).
        psums = []
        off = 0
        while off < NFMAX:
            w = min(512, NFMAX - off)
            ps = psum_pool.tile([1, w], f32, name=f"ps{off}", tag=f"ps{off}")
            nc.vector.memset(ps[:], 0.0)
            psums.append((off, w, ps))
            off += w
        last_user = {}
        for i, C in enumerate(tiles):
            for k, (poff, w, ps) in enumerate(psums):
                if NPAIR * C > poff:
                    last_user[k] = i

        b0 = 0
        for i, C in enumerate(tiles):
            FD = C * F
            NF = NPAIR * C
            rows = P * C
            xv = x_ext[b0 : b0 + rows, :].rearrange("(p c) f -> p (c f)", p=P)
            gv = g_ext[b0 : b0 + rows, :].rearrange("(p c) f -> p (c f)", p=P)
            b0 += rows

            xt = ins_pool.tile([P, FD], f32, tag="xin")
            gt = ins_pool.tile([P, FD], f32, tag="gin")
            # one full-tile DMA per tensor: HWDGE FIFOs serialize per-DMA
            # completion latency, so fewer+bigger wins
            nc.sync.dma_start(out=xt[:], in_=xv)
            nc.sync.dma_start(out=gt[:], in_=gv)

            # compact uv-only bf16 copies into ONE combined tile:
            # pred batch rows [0:C), gt rows [C:2C). Bone subs and the
            # square then process both tensors in single ops (half the
            # per-op fixed cost at the same 2x rate).
            uc = mid_pool.tile([P, 2 * C, 21, 2], bf16, tag="uc")
            _compact_uv(nc, xt, uc, C, 0, C)
            _compact_uv(nc, gt, uc, C, C, 2 * C)
            dc = mid_pool.tile([P, 2 * C, NPAIR, 2], bf16, tag="dc")
            _emit_bone_subs(nc, uc, dc, 2 * C)

            prod = mid_pool.tile([P, C, NPAIR, 2], bf16, tag="prod")
            nc.vector.tensor_mul(
                out=prod[:].rearrange("p c q x -> p (c q x)"),
                in0=dc[:, 0:C, :, :].rearrange("p c q x -> p (c q x)"),
                in1=dc[:, C : 2 * C, :, :].rearrange("p c q x -> p (c q x)"),
            )
            sqc = mid_pool.tile([P, 2 * C, NPAIR, 2], bf16, tag="sqc")
            nc.scalar.activation(
                out=sqc[:].rearrange("p c q x -> p (c q x)"),
                in_=dc[:].rearrange("p c q x -> p (c q x)"),
                func=AF.Square,
            )

            dot = small_pool.tile([P, C, NPAIR], bf16, tag="dot")
            nc.vector.tensor_add(
                out=dot[:], in0=prod[:, :, :, 0], in1=prod[:, :, :, 1]
            )
            # n1 (pred, rows 0:C) and n2 (gt, rows C:2C) in one add
            nc2 = small_pool.tile([P, 2 * C, NPAIR], bf16, tag="nc2")
            nc.vector.tensor_add(
                out=nc2[:], in0=sqc[:, :, :, 0], in1=sqc[:, :, :, 1]
            )

            den = small_pool.tile([P, NF], bf16, tag="den")
            nc.vector.tensor_mul(
                out=den[:],
                in0=nc2[:, 0:C, :].rearrange("p c q -> p (c q)"),
                in1=nc2[:, C : 2 * C, :].rearrange("p c q -> p (c q)"),
            )

            # t = |dot| * den^-1/2 = |dot| * exp(-0.5*ln(den));
            # ACT Rsqrt is banned (accuracy), Ln/Exp/Abs/Square share one table set
            a = small_pool.tile([P, NF], bf16, tag="a")
            nc.scalar.activation(
                out=a[:], in_=dot[:].rearrange("p c q -> p (c q)"), func=AF.Abs
            )
            lg = small_pool.tile([P, NF], bf16, tag="lg")
            nc.scalar.activation(out=lg[:], in_=den[:], func=AF.Ln, bias=eps[:])
            e = small_pool.tile([P, NF], bf16, tag="e")
            nc.scalar.activation(out=e[:], in_=lg[:], func=AF.Exp, scale=-0.5)
            t = small_pool.tile([P, NF], bf16, tag="t")
            nc.vector.tensor_mul(out=t[:], in0=a[:], in1=e[:])

            for k, (poff, w, ps) in enumerate(psums):
                if NF <= poff:
                    continue
                ww = min(w, NF - poff)
                nc.tensor.matmul(
                    out=ps[:, 0:ww],
                    lhsT=ones[:],
                    rhs=t[:, poff : poff + ww],
                    start=False,
                    stop=(last_user[k] == i),
                    skip_group_check=True,
                )

        # Tail: reduce each PSUM bank directly (DVE reads PSUM), then the
        # tiny per-bank sums, then DMA the scalar out
        t3 = const_pool.tile([1, len(psums)], f32)
        for k, (poff, w, ps) in enumerate(psums):
            nc.vector.tensor_reduce(
                out=t3[:, k : k + 1],
                in_=ps[:],
                op=mybir.AluOpType.add,
                axis=mybir.AxisListType.X,
            )
        total = const_pool.tile([1, 1], f32)
        nc.vector.tensor_reduce(
            out=total[:], in_=t3[:], op=mybir.AluOpType.add, axis=mybir.AxisListType.X
        )
        nc.sync.dma_start(out=out_ext[:], in_=total[:])

    return nc


_NC_CACHE: dict = {}

DEFAULT_TILES = (32, 32, 64, 64, 64, 64, 64, 64, 64)


def _get_nc(tiles) -> bass.Bass:
    key = tuple(tiles)
    if key not in _NC_CACHE:
        nc = build_nc(list(tiles))
        _split_excess_waits(nc)
        _NC_CACHE[key] = nc
    return _NC_CACHE[key]


def kernel(jt_uvd_pred, jt_uvd_gt, _tiles=DEFAULT_TILES, _trace: bool = False):
    pred = np.ascontiguousarray(np.asarray(jt_uvd_pred), dtype=np.float32)
    gt = np.ascontiguousarray(np.asarray(jt_uvd_gt), dtype=np.float32)
    Btot = pred.shape[0]
    assert pred.shape == (Btot, J, DCOORD) and gt.shape == (Btot, J, DCOORD)
    bl = P * sum(_tiles)
    assert bl * NCORES == Btot, (Btot, _tiles)

    nc = _get_nc(_tiles)
    in_maps = []
    for c in range(NCORES):
        sl = slice(c * bl, (c + 1) * bl)
        in_maps.append(
            {
                "jt_uvd_pred": pred[sl].reshape(bl, F),
                "jt_uvd_gt": gt[sl].reshape(bl, F),
            }
        )
    res = run_bass_kernel_spmd(
        nc, in_maps, core_ids=list(range(NCORES)), trace=_trace
    )
    total = sum(float(res.results[i]["out"][0, 0]) for i in range(NCORES))
    loss = 1.0 - total / (Btot * NPAIR)
    out = np.float32(loss)
    if _trace:
        return out, res
    return out

